# revision 21
# baseline (speedup 1.0000x reference)
"""Multi-head attention (RMSNorm-QK + RoPE + softmax + proj) on 8 Trainium2 cores.

Sharding: core c handles batch b = c//4 and heads [3*(c%4), 3*(c%4)+3).
Each core computes qkv for its heads, flash-style attention, and a partial
projection over its heads' channels; the host sums the 4 partials per batch.

Layout tricks (all fp32, matmuls in float32r at 1 cyc/row):
 - q^T/k^T layout [head_dim, tokens]; head-dim rows permuted so the RoPE
   half-swap is an intra-quadrant stream_shuffle.
 - RMS-norm: sum(q^2) via ones-pair matmul; rsqrt = exp(-0.5*ln(x)) so the
   whole kernel uses one ACT table set (natural_log_exp_and_others).
 - softmax without max-subtraction (logits bounded by RMS norm); denominators
   via an appended ones-column in the PV matmul; 1/denom on DVE.
 - qkv/proj biases via K=1 matmul rows.
"""
import sys

for _p in ("/opt/trn_rl_repo", "/opt/trn_rl_repo/concourse"):
    if _p not in sys.path:
        sys.path.insert(0, _p)

import numpy as np
from contextlib import ExitStack

import concourse.bass as bass
import concourse.tile as tile
import concourse.mybir as mybir
from concourse.bass_utils import run_bass_kernel_spmd

F32 = mybir.dt.float32
F32R = mybir.dt.float32r
AF = mybir.ActivationFunctionType

B, N, C = 2, 2048, 768
H, HD = 12, 64
HP = 3            # heads per core
NCORES = 8
CCH = C // 128    # 6 contraction chunks
NT = N // 512     # 4 token tiles of 512
KB = N // 128     # 16 k-blocks of 128
EPS = 1e-6

SWAP_MASK = [(i + 16) % 32 for i in range(32)]
# head-dim permutation: pair-exchange (d <-> d+32) becomes intra-quadrant
PERM = np.concatenate([np.arange(0, 16), np.arange(32, 48),
                       np.arange(16, 32), np.arange(48, 64)])
SIGN = np.where(PERM < 32, -1.0, 1.0).astype(np.float32)

_NC_CACHE = {}


def build_nc(split_waits=True):
    nc = bass.Bass(target_bir_lowering=True)
    xT = nc.declare_dram_parameter("xT", [C, N], F32R, isOutput=False)
    wqk = nc.declare_dram_parameter("wqk", [C, HP * 128], F32R, isOutput=False)
    wv = nc.declare_dram_parameter("wv", [C, 256], F32R, isOutput=False)
    bqk = nc.declare_dram_parameter("bqk", [1, HP * 128], F32R, isOutput=False)
    bv = nc.declare_dram_parameter("bv", [1, 256], F32R, isOutput=False)
    cos2w = nc.declare_dram_parameter("cos2w", [128, N], F32, isOutput=False)
    sinSw = nc.declare_dram_parameter("sinSw", [128, N], F32, isOutput=False)
    sel4 = nc.declare_dram_parameter("sel4", [128, 512], F32R, isOutput=False)
    wp = nc.declare_dram_parameter("wp", [HP * HD, C], F32R, isOutput=False)
    onesd = nc.declare_dram_parameter("onesd", [128, 512], F32R, isOutput=False)
    onespd = nc.declare_dram_parameter("onespd", [128, 2], F32R, isOutput=False)
    vones = nc.declare_dram_parameter("vones", [128, HP * KB], F32R, isOutput=False)
    out = nc.declare_dram_parameter("out", [N, C], F32, isOutput=True)

    with tile.TileContext(nc) as tc, ExitStack() as ctx:
        sb = ctx.enter_context(tc.tile_pool(name="sb", bufs=1))
        tp = ctx.enter_context(tc.tile_pool(name="tp", bufs=2))
        pe = ctx.enter_context(tc.tile_pool(name="pe", bufs=3))   # pexp
        tp1 = ctx.enter_context(tc.tile_pool(name="tp1", bufs=1))
        fps = ctx.enter_context(tc.tile_pool(name="fps", bufs=2, space="PSUM"))
        sA = ctx.enter_context(tc.tile_pool(name="sA", bufs=1, space="PSUM"))
        sB = ctx.enter_context(tc.tile_pool(name="sB", bufs=1, space="PSUM"))
        oA = ctx.enter_context(tc.tile_pool(name="oA", bufs=1, space="PSUM"))
        oB = ctx.enter_context(tc.tile_pool(name="oB", bufs=1, space="PSUM"))


        # ---------- prologue: loads + consts ----------
        xs = []
        for c in range(CCH):
            t = sb.tile([128, N], F32R, tag=f"x{c}")
            nc.sync.dma_start(t[:, 0:1024], xT[c * 128:(c + 1) * 128, 0:1024])
            nc.sync.dma_start(t[:, 1024:2048], xT[c * 128:(c + 1) * 128, 1024:2048])
            xs.append(t)
        wqk_sb, wv_sb = [], []
        for c in range(CCH):
            t = sb.tile([128, HP * 128], F32R, tag=f"wqk{c}")
            nc.sync.dma_start(t[:], wqk[c * 128:(c + 1) * 128, :])
            wqk_sb.append(t)
            t = sb.tile([128, 256], F32R, tag=f"wv{c}")
            nc.gpsimd.dma_start(t[:], wv[c * 128:(c + 1) * 128, :])
            wv_sb.append(t)
        bqk_sb = sb.tile([1, HP * 128], F32R, tag="bqk")
        nc.sync.dma_start(bqk_sb[:], bqk[:, :])
        bv_sb = sb.tile([1, 256], F32R, tag="bv")
        nc.gpsimd.dma_start(bv_sb[:], bv[:, :])
        cos_sb = sb.tile([128, N], F32, tag="cos")
        nc.gpsimd.dma_start(cos_sb[:], cos2w[:, :])
        sin_sb = sb.tile([128, N], F32, tag="sin")
        nc.gpsimd.dma_start(sin_sb[:], sinSw[:, :])
        sel_sb = sb.tile([128, 512], F32R, tag="sel")
        nc.gpsimd.dma_start(sel_sb[:], sel4[:, :])
        wp0_sb = sb.tile([128, C], F32R, tag="wp0")
        nc.gpsimd.dma_start(wp0_sb[:], wp[0:128, :])
        wp1_sb = sb.tile([64, C], F32R, tag="wp1")
        nc.gpsimd.dma_start(wp1_sb[:], wp[128:192, :])

        ones_row = sb.tile([1, 512], F32R, tag="ones_row")
        nc.gpsimd.dma_start(ones_row[:], onesd[0:1, :])
        onesp = sb.tile([128, 2], F32R, tag="onesp")
        nc.gpsimd.dma_start(onesp[:], onespd[:, :])
        ones64 = sb.tile([1, 64], F32R, tag="ones64")
        nc.gpsimd.dma_start(ones64[:], onesd[0:1, 0:64])
        eps_t = sb.tile([128, 1], F32, tag="eps")
        nc.gpsimd.memset(eps_t[:], EPS)
        v3i = sb.tile([128, HP * KB * 65], F32R, tag="v3i")  # [v_h(kb) | 1] blocks
        nc.gpsimd.dma_start(
            v3i[:].rearrange("p (b n) -> p b n", n=65)[:, :, 64:65],
            vones[:, :, None])

        # qT/kT packed by head pairs so S-matmul operands share a base partition
        q12 = sb.tile([128, N], F32R, tag="q12")   # qT(0) rows 0:64, qT(1) rows 64:128
        k12 = sb.tile([128, N], F32R, tag="k12")
        q3 = sb.tile([64, N], F32R, tag="q3")
        k3 = sb.tile([64, N], F32R, tag="k3")

        def qT(h):
            return (q12[0:64], q12[64:128], q3[:])[h]

        def kT(h):
            return (k12[0:64], k12[64:128], k3[:])[h]

        oall_a = sb.tile([128, N], F32R, tag="oall_a")   # heads 0,1 O^T
        oall_b = sb.tile([64, N], F32R, tag="oall_b")    # head 2 O^T
        t4_all = sb.tile([128, N], F32, tag="t4_all")
        s_sb = sb.tile([128, 512], F32, tag="s_sb")
        nc.gpsimd.memset(s_sb[:], 1.0)
        lnv = sb.tile([128, 512], F32, tag="lnv")
        sv = sb.tile([128, 512], F32R, tag="sv")

        def mm(out_ap, lhsT, rhs, start, stop):
            nc.tensor.matmul(out_ap, lhsT.bitcast(F32R), rhs.bitcast(F32R),
                             start=start, stop=stop, skip_group_check=True)

        # ---------- qkv for head h ----------
        def qkv(h):
            for t in range(NT):
                ts = slice(t * 512, (t + 1) * 512)
                qk_ps = fps.tile([128, 512], F32, tag="flex")
                for c in range(CCH):
                    mm(qk_ps[:], wqk_sb[c][:, h * 128:(h + 1) * 128],
                       xs[c][:, ts], c == 0, False)
                mm(qk_ps[:], bqk_sb[:, h * 128:(h + 1) * 128], ones_row[:],
                   False, True)
                t1 = tp1.tile([128, 512], F32, tag="t1")
                nc.vector.tensor_mul(t1[:], qk_ps[:], cos_sb[:, ts])
                t2 = tp.tile([128, 512], F32, tag="t2")
                nc.vector.stream_shuffle(t2[:], qk_ps[:], SWAP_MASK)
                sq = tp.tile([128, 512], F32R, tag="sq")
                nc.vector.tensor_mul(sq[:], t2[:], t2[:])
                t3 = tp1.tile([128, 512], F32, tag="t3")
                nc.vector.tensor_mul(t3[:], t2[:], sin_sb[:, ts])
                sums_ps = fps.tile([2, 512], F32, tag="flex")
                mm(sums_ps[:], onesp[:], sq[:], True, True)
                nc.vector.tensor_copy(s_sb[32 * t:32 * t + 2, :], sums_ps[:])
                nc.vector.tensor_add(t4_all[:, ts], t1[:], t3[:])
            nc.scalar.activation(lnv[:], s_sb[:], AF.Ln,
                                 bias=eps_t[:], scale=1.0 / HD)
            nc.scalar.activation(sv[:], lnv[:], AF.Exp, bias=0.0, scale=-0.5)
            for t in range(NT):
                ts = slice(t * 512, (t + 1) * 512)
                sqk_ps = fps.tile([128, 512], F32, tag="flex")
                mm(sqk_ps[:], sel_sb[:, t * 128:(t + 1) * 128], sv[:],
                   True, True)
                nc.vector.tensor_mul(qT(h)[:, ts], t4_all[0:64, ts],
                                     sqk_ps[0:64, :])
                nc.vector.tensor_mul(kT(h)[:, ts], t4_all[64:128, ts],
                                     sqk_ps[64:128, :])

        # ---------- v for all heads ----------
        def vphase():
            for tt in range(KB):
                v_ps = fps.tile([128, 256], F32, tag="flex")
                for c in range(CCH):
                    mm(v_ps[:], xs[c][:, tt * 128:(tt + 1) * 128], wv_sb[c][:],
                       c == 0, False)
                mm(v_ps[:], ones_row[0:1, 0:128], bv_sb[:], False, True)
                # strided copy of 3 head-blocks into v3i (+ ones col at 64)
                dst = v3i[:].rearrange("p (h k n) -> p h k n", h=HP, k=KB)
                nc.vector.tensor_copy(
                    dst[:, :, tt, 0:64],
                    v_ps[:, 0:192].rearrange("p (h n) -> p h n", h=HP))

        # ---------- attention ----------
        # 16 k-blocks in groups of 2 (one 2-bank PSUM tile per group)
        G2 = [(2 * g, 2 * g + 1) for g in range(8)]

        def epilogue(h, qt, o_ps):
            qs = slice(qt * 512, (qt + 1) * 512)
            ld = tp1.tile([1, 512], F32, tag="ld")
            nc.scalar.activation(ld[:], o_ps[64:65, :], AF.Ln,
                                 bias=0.0, scale=1.0)
            rec = tp1.tile([1, 512], F32R, tag="rec")
            nc.scalar.activation(rec[:], ld[:], AF.Exp, bias=0.0, scale=-1.0)
            rec_ps = fps.tile([64, 512], F32, tag="flex")
            mm(rec_ps[:], ones64[:], rec[:], True, True)
            rec_b = tp1.tile([64, 512], F32, tag="rec_b")
            nc.vector.tensor_copy(rec_b[:], rec_ps[:])
            if h < 2:
                dst = oall_a[h * 64:(h + 1) * 64, qs]
            else:
                dst = oall_b[:, qs]
            nc.vector.tensor_mul(dst, o_ps[0:64, :], rec_b[:])

        def smm(spool, h, kbs, qs):
            s_ps = spool.tile([128, 1024], F32, tag="s")
            for j, kb in enumerate(kbs):
                mm(s_ps[:, j * 512:(j + 1) * 512],
                   kT(h)[:, kb * 128:(kb + 1) * 128], qT(h)[:, qs], True, True)
            return s_ps

        def pexp_of(s_ps):
            px = pe.tile([128, 1024], F32R, tag="pexp")
            nc.scalar.activation(px[:], s_ps[:], AF.Exp, bias=0.0, scale=0.125)
            return px

        def omm(o_ps, h, kbs, px):
            for j, kb in enumerate(kbs):
                mm(o_ps[:], v3i[:, (h * KB + kb) * 65:(h * KB + kb) * 65 + 65],
                   px[:, j * 512:(j + 1) * 512], kb == 0, kb == KB - 1)

        # ---------- partial projection (token tiles of one q-tile) ----------
        def proj_qt(qt):
            for tt in range(4 * qt, 4 * qt + 4):
                po = tp1.tile([128, C], F32, tag="po")
                for half in range(2):
                    cs = slice(half * 384, (half + 1) * 384)
                    p_ps = fps.tile([128, 512], F32, tag="flex")
                    mm(p_ps[:, 0:384], oall_a[:, tt * 128:(tt + 1) * 128],
                       wp0_sb[:, cs], True, False)
                    mm(p_ps[:, 0:384], oall_b[:, tt * 128:(tt + 1) * 128],
                       wp1_sb[:, cs], False, True)
                    nc.vector.tensor_copy(po[:, cs], p_ps[:, 0:384])
                nc.sync.dma_start(out[tt * 128:(tt + 1) * 128, :], po[:])


        def attn_single(h):
            for qt in range(NT):
                qs = slice(qt * 512, (qt + 1) * 512)
                o_ps = oA.tile([65, 512], F32, tag="o")
                for g, kbs in enumerate(G2):
                    s_ps = smm(sA if g % 2 == 0 else sB, h, kbs, qs)
                    px = pexp_of(s_ps)
                    omm(o_ps, h, kbs, px)
                epilogue(h, qt, o_ps)

        def attn_pair(h0, h1):
            # h0/h1 S-matmuls sit in different PE row-groups (base partition
            # 0 vs 64) and different PSUM banks -> they run concurrently.
            for qt in range(NT):
                qs = slice(qt * 512, (qt + 1) * 512)
                o0 = oA.tile([65, 512], F32, tag="o")
                o1 = oB.tile([65, 512], F32, tag="o")
                for kbs in G2:
                    s0 = smm(sA, h0, kbs, qs)
                    s1 = smm(sB, h1, kbs, qs)
                    px0 = pexp_of(s0)
                    omm(o0, h0, kbs, px0)
                    px1 = pexp_of(s1)
                    omm(o1, h1, kbs, px1)
                epilogue(h0, qt, o0)
                epilogue(h1, qt, o1)
                proj_qt(qt)

        qkv(0)
        vphase()
        attn_single(0)
        qkv(1)
        qkv(2)
        attn_pair(1, 2)

    if split_waits:
        _split_waits(nc)
    return nc


def _split_waits(nc):
    """This walrus build lowers at most one sync-wait per instruction (the
    matmul LDW struct rejects 2+). Move excess waits onto NoOps inserted
    just before, on the same engine queue — queues are in-order, so the
    constraint is preserved exactly."""
    k = 0
    for fn in nc.m.functions:
        for bb in fn.blocks:
            il = bb.instructions
            idx = 0
            while idx < len(il):
                inst = il[idx]
                si = inst.sync_info
                eng = getattr(inst, "engine", None)
                if (si is not None and len(si.on_wait) > 1
                        and eng is not None
                        and str(eng) != "EngineType.Unassigned"):
                    waits = list(si.on_wait)
                    inst.sync_info = mybir.SyncInfo(
                        on_wait=[waits[-1]], on_update=list(si.on_update))
                    for w in waits[:-1]:
                        nop = mybir.InstNoOp(
                            name=f"I-waitnop-{k}", engine=eng, ins=[], outs=[],
                            sync_info=mybir.SyncInfo(on_wait=[w], on_update=[]))
                        k += 1
                        il.insert(idx, nop)
                        idx += 1
                idx += 1


def _prep_core_inputs(core, x, rope_cos, rope_sin, qkv_kernel, qkv_bias,
                      proj_kernel, proj_bias, q_norm_w, k_norm_w):
    b = core // 4
    heads = [3 * (core % 4) + i for i in range(HP)]

    wq = qkv_kernel.reshape(C, 3, H, HD)
    bq = qkv_bias.reshape(3, H, HD)

    xT = np.ascontiguousarray(x[b].T, dtype=np.float32)

    wqk = np.empty((C, HP * 128), np.float32)
    bqk = np.empty((1, HP * 128), np.float32)
    for i, h in enumerate(heads):
        wqk[:, i * 128:i * 128 + 64] = wq[:, 0, h, PERM]
        wqk[:, i * 128 + 64:(i + 1) * 128] = wq[:, 1, h, PERM]
        bqk[0, i * 128:i * 128 + 64] = bq[0, h, PERM]
        bqk[0, i * 128 + 64:(i + 1) * 128] = bq[1, h, PERM]

    wv = np.zeros((C, 256), np.float32)
    bv = np.zeros((1, 256), np.float32)
    for i, h in enumerate(heads):
        wv[:, i * 64:(i + 1) * 64] = wq[:, 2, h, :]
        bv[0, i * 64:(i + 1) * 64] = bq[2, h, :]

    cosT = rope_cos.T  # (HD, N)
    sinT = rope_sin.T
    cos2w = np.empty((128, N), np.float32)
    sinSw = np.empty((128, N), np.float32)
    cos2w[0:64] = cosT[PERM] * q_norm_w[PERM][:, None]
    cos2w[64:128] = cosT[PERM] * k_norm_w[PERM][:, None]
    sinSw[0:64] = SIGN[:, None] * sinT[PERM] * q_norm_w[PERM][:, None]
    sinSw[64:128] = SIGN[:, None] * sinT[PERM] * k_norm_w[PERM][:, None]

    onesd = np.ones((128, 512), np.float32)
    onespd = np.zeros((128, 2), np.float32)
    onespd[0:64, 0] = 1.0    # col0: ones on q rows
    onespd[64:128, 1] = 1.0  # col1: ones on k rows
    vones = np.ones((128, HP * KB), np.float32)

    sel4 = np.zeros((128, 512), np.float32)
    for t in range(NT):
        sel4[32 * t, t * 128:t * 128 + 64] = 1.0
        sel4[32 * t + 1, t * 128 + 64:(t + 1) * 128] = 1.0

    rows = np.concatenate([np.arange(h * HD, (h + 1) * HD) for h in heads])
    wp = np.ascontiguousarray(proj_kernel[rows, :], dtype=np.float32)

    return {"xT": xT, "wqk": wqk, "wv": wv, "bqk": bqk, "bv": bv,
            "cos2w": cos2w, "sinSw": sinSw, "sel4": sel4,
            "wp": wp, "onesd": onesd, "onespd": onespd, "vones": vones}


def kernel(x, rope_cos, rope_sin, qkv_kernel, qkv_bias, proj_kernel,
           proj_bias, q_norm_w, k_norm_w, _trace=False):
    args = [np.asarray(a, dtype=np.float32) for a in
            (x, rope_cos, rope_sin, qkv_kernel, qkv_bias, proj_kernel,
             proj_bias, q_norm_w, k_norm_w)]
    in_maps = [_prep_core_inputs(c, *args) for c in range(NCORES)]

    if "nc" not in _NC_CACHE:
        _NC_CACHE["nc"] = build_nc()
    nc = _NC_CACHE["nc"]

    res = run_bass_kernel_spmd(nc, in_maps, core_ids=list(range(NCORES)),
                               trace=_trace)
    parts = [res.results[c]["out"] for c in range(NCORES)]
    out = np.empty((B, N, C), np.float32)
    pb = np.asarray(proj_bias, dtype=np.float32)
    for b in range(B):
        out[b] = parts[4 * b] + parts[4 * b + 1] + parts[4 * b + 2] + parts[4 * b + 3] + pb
    if _trace:
        kernel.last_results = res
    return out
